# revision 2
# baseline (speedup 1.0000x reference)
"""3-layer GCN forward pass on 8 TRN2 NeuronCores.

Strategy (vertex-cut graph parallelism):
  - Each core owns a contiguous block of N/8 destination nodes; edges are
    partitioned by destination so segment sums stay local.
  - Per layer, each GCN conv is computed as (A_hat @ x) @ W + b, i.e.
    aggregate first, then the dense matmul.
  - Prologue: x_tilde = dinv * x is computed per-core on its own slice and
    AllGathered; every layer then gathers x_tilde rows per edge with
    dma_gather (4 SWDGE queues in parallel) and segment-sums them on the
    tensor engine as one-hot matmuls accumulating z^T in PSUM (channels on
    partitions).  One-hot S matrices depend only on the graph; they are
    precomputed host-side and streamed from HBM.
  - The destination-side dinv is applied per-column after aggregation;
    self loops are plain edges, reproducing the h/deg self term exactly.
  - Between layers: the per-core slice of the new features is scaled,
    transposed back to node-major (DMA transpose) and AllGathered.
  - int16 gather indices only reach 32768 rows, so each core's edges are
    split into a "lo" stream (padded row id < 32768) and a "hi" stream,
    gathered from two base offsets; PSUM partials from the two passes are
    combined in SBUF.
"""

import sys
import types

sys.path.insert(0, "/opt/trn_rl_repo")

import numpy as np
import ml_dtypes

import concourse.bass as bass  # noqa: F401
import concourse.bacc as bacc
import concourse.tile as tile
import concourse.mybir as mybir
from concourse import bass_utils
from concourse.masks import make_identity

BF16 = ml_dtypes.bfloat16
FP32 = np.float32


def _install_ntff_hook():
    """The image's antenv lacks axon_hooks; shim it so trace=True works."""
    if "antenv.axon_hooks" in sys.modules:
        return
    mod = types.ModuleType("antenv.axon_hooks")
    mod._hook = None
    mod.set_axon_ntff_profile_hook = lambda h: setattr(mod, "_hook", h)
    mod.get_axon_ntff_profile_hook = lambda: mod._hook
    sys.modules["antenv.axon_hooks"] = mod
    try:
        import antenv

        antenv.axon_hooks = mod
        if "/root/.axon_site" not in sys.path:
            sys.path.insert(0, "/root/.axon_site")
        from trn_agent_boot.trn_boot import _ntff_profile_via_ctypes

        mod.set_axon_ntff_profile_hook(
            _ntff_profile_via_ctypes("/opt/axon/libaxon_pjrt.so"))
    except Exception:
        pass


class Cfg:
    def __init__(self, n=50000, c=128, hid=128, out_c=64, ncores=8,
                 lo_rows=32768, piece_ch=6, sbatch=16):
        self.N = n
        self.C = c
        self.HID = hid
        self.OUT_C = out_c
        self.NCORES = ncores
        self.NPC = n // ncores
        self.TPC = (self.NPC + 127) // 128
        self.NPC_PAD = self.TPC * 128
        self.NPAD = ncores * self.NPC_PAD
        self.LO_ROWS = lo_rows
        self.TA = (self.TPC + 1) // 2          # tiles in half A
        self.SPLITA = self.TA * 128            # per-rank rows in half A
        self.ROWSA = ncores * self.SPLITA
        self.ROWSB = ncores * (self.NPC_PAD - self.SPLITA)
        assert self.ROWSA <= 32768 and self.ROWSB <= 32768, \
            "half-buffers must stay int16-addressable"
        self.PIECE_CH = piece_ch
        self.SBATCH = sbatch
        self.SELF_EDGES = False  # True: self-loops gathered as plain edges
        assert n % ncores == 0


FULL = Cfg()


# ---------------------------------------------------------------- host prep
def _preprocess(cfg, edge_index):
    """Compute per-core gather indices and one-hot S blocks."""
    src = np.asarray(edge_index[0]).astype(np.int64)
    dst = np.asarray(edge_index[1]).astype(np.int64)
    n, npc, npcp, tpc = cfg.N, cfg.NPC, cfg.NPC_PAD, cfg.TPC

    deg = (np.bincount(dst, minlength=n) + 1.0).astype(np.float64)
    dinv = (1.0 / np.sqrt(deg)).astype(np.float32)

    # self-loops are handled on-chip (z += x_tilde before the dst scale),
    # so only the real edges are gathered.
    if cfg.SELF_EDGES:
        loop = np.arange(n, dtype=np.int64)
        allsrc = np.concatenate([src, loop])
        alldst = np.concatenate([dst, loop])
    else:
        allsrc = src
        alldst = dst

    src_r = allsrc // npc
    src_i = allsrc % npc
    bucket = (src_i >= cfg.SPLITA).astype(np.int64)
    szB = npcp - cfg.SPLITA
    ta, tb = cfg.TA, cfg.TPC - cfg.TA
    # table rows are p-major within each core block (row = p*T + t) so the
    # AllGather input write is a contiguous copy of the SBUF tile
    iA = src_i
    iB = src_i - cfg.SPLITA
    m = np.where(bucket == 0,
                 src_r * cfg.SPLITA + (iA % 128) * ta + iA // 128,
                 src_r * szB + (iB % 128) * tb + iB // 128)
    owner = alldst // npc
    local = alldst - owner * npc
    tile_id = local // 128
    pos = local % 128

    order = np.lexsort((m, bucket, tile_id, owner))
    m_s = m[order]
    owner_s = owner[order]
    tile_s = tile_id[order]
    bucket_s = bucket[order]
    pos_s = pos[order]

    counts = np.zeros((cfg.NCORES, tpc, 2), dtype=np.int64)
    np.add.at(counts, (owner_s, tile_s, bucket_s), 1)
    clo = np.ceil(counts[:, :, 0] / 128).astype(int).max(axis=0)  # [tpc]
    chi = np.ceil(counts[:, :, 1] / 128).astype(int).max(axis=0)

    flat_counts = counts.reshape(-1)
    flat_starts = np.concatenate([[0], np.cumsum(flat_counts)[:-1]])
    starts = flat_starts.reshape(cfg.NCORES, tpc, 2)

    nch = int(clo.sum() + chi.sum())
    l_lo = max(int(clo.sum()) * 128, 16)
    l_hi = max(int(chi.sum()) * 128, 16)

    per_core = []
    for c in range(cfg.NCORES):
        idx_lo = np.zeros(l_lo, dtype=np.int32)
        idx_hi = np.zeros(l_hi, dtype=np.int32)
        dpos = np.full((nch, 128), 255, dtype=np.int32)

        ch_g = 0
        for b, cc, idx_arr in ((0, clo, idx_lo), (1, chi, idx_hi)):
            off = 0
            for t in range(tpc):
                s0 = starts[c, t, b]
                cnt = counts[c, t, b]
                cap = cc[t] * 128
                idx_arr[off:off + cnt] = m_s[s0:s0 + cnt]
                blk = dpos[ch_g:ch_g + cc[t]].reshape(-1)
                blk[:cnt] = pos_s[s0:s0 + cnt]
                off += cap
                ch_g += cc[t]
        assert ch_g == nch

        def wrap(stream):
            # idx j consumed from [j % 16, j // 16]; replicate to 8 groups
            a = stream.reshape(-1, 16).T.astype(np.int16)
            return np.tile(a, (8, 1)).copy()

        nblk_s = (nch + cfg.SBATCH - 1) // cfg.SBATCH
        dpos_pad = np.full((nblk_s * cfg.SBATCH, 128), 255, dtype=np.int32)
        dpos_pad[:nch] = dpos
        # one-hot S blocks are generated on-chip from dpos (edge -> dst
        # position); only the positions are shipped: [128 edge, nch] bf16.
        dpos_t = np.ascontiguousarray(dpos_pad.T).astype(BF16)

        dinv_own = np.zeros(npcp, dtype=np.float32)
        dinv_own[:npc] = dinv[c * npc:(c + 1) * npc]
        per_core.append({
            "idx_lo": wrap(idx_lo),
            "idx_hi": wrap(idx_hi),
            "dpos_t": dpos_t,
            "dinv_bc": np.tile(dinv_own, (128, 1)).astype(BF16),
        })

    plan = {
        "clo": clo.tolist(),
        "chi": chi.tolist(),
        "nch": nch,
        "nblk_s": nblk_s,
        "l_lo": l_lo,
        "l_hi": l_hi,
    }
    return plan, per_core, dinv


# ------------------------------------------------------------- bass program
def _build(cfg, plan):
    clo, chi = plan["clo"], plan["chi"]
    nch, l_lo, l_hi = plan["nch"], plan["l_lo"], plan["l_hi"]
    tpc, npcp, npad = cfg.TPC, cfg.NPC_PAD, cfg.NPAD
    C, OUT_C = cfg.C, cfg.OUT_C
    bf = mybir.dt.bfloat16
    f32 = mybir.dt.float32

    nc = bacc.Bacc("TRN2", target_bir_lowering=False, debug=False,
                   num_devices=cfg.NCORES, num_swdge_queues=4,
                   dynamic_dma_scratch_size=65536)

    # layer-0 gather tables (x_tilde halves, replicated) and the
    # channel-major own-slice x_tilde^T are precomputed host-side
    xt0A_d = nc.dram_tensor("xt0A", [cfg.ROWSA, C], bf,
                            kind="ExternalInput")
    xt0B_d = nc.dram_tensor("xt0B", [cfg.ROWSB, C], bf,
                            kind="ExternalInput")
    xtprev0_d = nc.dram_tensor("xtprev0", [128, npcp], bf,
                               kind="ExternalInput")
    w_d = [nc.dram_tensor(f"w{i}", [C, C if i < 2 else OUT_C], bf,
                          kind="ExternalInput") for i in range(3)]
    b_d = [nc.dram_tensor(f"b{i}", [C if i < 2 else OUT_C, 1], f32,
                          kind="ExternalInput") for i in range(3)]
    idxlo_d = nc.dram_tensor("idx_lo", [128, max(l_lo // 16, 1)],
                             mybir.dt.int16, kind="ExternalInput")
    idxhi_d = nc.dram_tensor("idx_hi", [128, max(l_hi // 16, 1)],
                             mybir.dt.int16, kind="ExternalInput")
    nblk_s = plan["nblk_s"]
    dpos_d = nc.dram_tensor("dpos_t", [128, nblk_s * cfg.SBATCH], bf,
                            kind="ExternalInput")
    iota_d = nc.dram_tensor("iota_bc", [128, 128], bf, kind="ExternalInput")
    idxdum_d = nc.dram_tensor("idx_dummy", [128, 8], mybir.dt.int16,
                              kind="ExternalInput")
    dinvbc_d = nc.dram_tensor("dinv_bc", [128, npcp], bf,
                              kind="ExternalInput")
    out_d = nc.dram_tensor("out", [cfg.NPC, OUT_C], f32,
                           kind="ExternalOutput")

    with tile.TileContext(nc) as tc:
        with (
            tc.tile_pool(name="const", bufs=1) as cpool,
            tc.tile_pool(name="g", bufs=8) as gpool,
            tc.tile_pool(name="s", bufs=4) as spool,
            tc.tile_pool(name="z", bufs=13) as zpool,
            tc.tile_pool(name="zs", bufs=2) as zspool,
            tc.tile_pool(name="xt", bufs=2) as xtpool,
            tc.tile_pool(name="nm", bufs=2) as nmpool,
            tc.tile_pool(name="fin", bufs=1) as finpool,
            tc.tile_pool(name="psA", bufs=5, space="PSUM") as psa,
            tc.tile_pool(name="psW", bufs=2, space="PSUM") as psw_pool,
            tc.tile_pool(name="psT", bufs=1, space="PSUM") as pst,
            tc.tile_pool(name="dram", bufs=1, space="DRAM") as dpool,
        ):
            # ---- constants into SBUF
            w_sb, b_sb = [], []
            for i in range(3):
                w = cpool.tile([C, C if i < 2 else OUT_C], bf, name=f"wt{i}")
                nc.sync.dma_start(w[:], w_d[i][:])
                bt = cpool.tile([C if i < 2 else OUT_C, 1], f32,
                                name=f"bt{i}")
                nc.sync.dma_start(bt[:], b_d[i][:])
                w_sb.append(w)
                b_sb.append(bt)
            idxlo_sb = cpool.tile([128, max(l_lo // 16, 1)], mybir.dt.int16,
                                  tag="idxlo")
            nc.sync.dma_start(idxlo_sb[:], idxlo_d[:])
            idxhi_sb = cpool.tile([128, max(l_hi // 16, 1)], mybir.dt.int16,
                                  tag="idxhi")
            nc.sync.dma_start(idxhi_sb[:], idxhi_d[:])
            dinvbc_sb = cpool.tile([128, npcp], bf, tag="dinvbc")
            nc.sync.dma_start(dinvbc_sb[:], dinvbc_d[:])
            dpos_sb = cpool.tile([128, nblk_s * cfg.SBATCH], bf, tag="dpos")
            nc.sync.dma_start(dpos_sb[:], dpos_d[:])
            ident = cpool.tile([OUT_C, OUT_C], f32, tag="ident")
            make_identity(nc, ident[:])
            # iota[p, j] = j, used to expand dpos into one-hot S blocks
            iota_sb = cpool.tile([128, 128], bf, tag="iota")
            nc.sync.dma_start(iota_sb[:], iota_d[:])

            ta, spA = cfg.TA, cfg.SPLITA
            szB = npcp - spA
            ag_inA = [None] + [dpool.tile([spA, C], bf, name=f"ag_inA{i}")
                               for i in (1, 2)]
            ag_inB = [None] + [dpool.tile([szB, C], bf, name=f"ag_inB{i}")
                               for i in (1, 2)]
            xt_fullA = [xt0A_d] + [
                dpool.tile([cfg.ROWSA, C], bf, name=f"xt_fullA{i}",
                           addr_space="Shared")
                for i in (1, 2)]
            xt_fullB = [xt0B_d] + [
                dpool.tile([cfg.ROWSB, C], bf, name=f"xt_fullB{i}",
                           addr_space="Shared")
                for i in (1, 2)]

            def emit_ag_half(li, half, nm_ap):
                buf_in = (ag_inA, ag_inB)[half][li]
                buf_out = (xt_fullA, xt_fullB)[half][li]
                nt = ta if half == 0 else tpc - ta
                nc.sync.dma_start(
                    buf_in[:].rearrange("(p t) c -> p t c", t=nt), nm_ap)
                nc.gpsimd.collective_compute(
                    "AllGather", mybir.AluOpType.bypass,
                    replica_groups=[list(range(cfg.NCORES))],
                    ins=[buf_in.opt()], outs=[buf_out.opt()],
                )

            # warm the gpsimd extended-isa library while the prologue runs
            dummy_src = dpool.tile([128, C], bf, name="dummy_src")
            nc.sync.dma_start(dummy_src[:], iota_sb[:])
            idxdum_sb = cpool.tile([128, 8], mybir.dt.int16, tag="idxdum")
            nc.sync.dma_start(idxdum_sb[:], idxdum_d[:])
            dummy_g = gpool.tile([128, 4, C], bf, tag="g")
            for q in range(4):
                nc.gpsimd.dma_gather(
                    dummy_g[:, q:q + 1, :], dummy_src[:],
                    idxdum_sb[:], 128, 128, C,
                    single_packet=True, queue_num=q)

            # ---- prologue: layer-0 x_tilde tables are host-precomputed;
            # just load the channel-major own-slice x_tilde^T (self term)
            xtprev0 = xtpool.tile([128, npcp], bf, tag="xt")
            nc.sync.dma_start(xtprev0[:], xtprev0_d[:])
            xt_prev = xtprev0

            lo_total, hi_total = sum(clo), sum(chi)
            # dst tiles 0..TSPLIT-1 = z blocks 0..6 cover the A half of xt;
            # the hi gather stream is broken there so the A-half AllGather
            # trigger can sit early in the gpsimd instruction stream.
            nzb = (npcp + 511) // 512
            TSPLIT = min(7 * 4, tpc)
            BSPLIT = min(7, nzb)
            hi_split = sum(chi[:TSPLIT])
            qrr = [0]
            gi = [0]

            for layer in range(3):
                cout = C if layer < 2 else OUT_C
                g_slots = []
                s_slots = []
                gi[0] = 0
                qrr[0] = 0

                def emit_gathers(idx_sb, src_dram, c0, c1):
                    c = c0
                    while c < c1:
                        pch = min(cfg.PIECE_CH, c1 - c)
                        g = gpool.tile([128, cfg.PIECE_CH, C], bf, tag="g")
                        nc.gpsimd.dma_gather(
                            g[:, :pch, :], src_dram,
                            idx_sb[:, c * 8:(c + pch) * 8],
                            pch * 128, pch * 128, C,
                            single_packet=True, queue_num=qrr[0] % 4)
                        qrr[0] += 1
                        for k in range(pch):
                            g_slots.append((g, k))
                        c += pch

                zb = [zpool.tile([128, 512], f32, tag="z", name=f"zb{i}")
                      for i in range(nzb)]

                def zsl(t):
                    return zb[t // 4][:, (t % 4) * 128:(t % 4) * 128 + 128]

                def emit_mm(phase, cc, t0, t1):
                    for t in range(t0, t1):
                        cnt = cc[t]
                        if cnt == 0:
                            if phase == 0 and chi[t] == 0:
                                nc.vector.memset(zsl(t), 0.0)
                            continue
                        ps = psa.tile([128, 128], f32, tag="psA")
                        for k in range(cnt):
                            g, gk = g_slots[gi[0]]
                            s, sk = s_slots[gi[0]]
                            gi[0] += 1
                            nc.tensor.matmul(ps[:], g[:, gk, :],
                                             s[:, sk, :],
                                             start=(k == 0),
                                             stop=(k == cnt - 1))
                        if phase == 0 or clo[t] == 0:
                            nc.scalar.copy(zsl(t), ps[:])
                        else:
                            nc.vector.tensor_add(zsl(t), zsl(t), ps[:])

                if layer < 2:
                    xt = xtpool.tile([128, npcp], bf, tag="xt")
                else:
                    fin = finpool.tile([OUT_C, npcp], f32, tag="fin")
                    out_nm = finpool.tile([128, tpc, OUT_C], f32,
                                          tag="onm")
                nblk = [(i * 512, min(512, npcp - i * 512))
                        for i in range(nzb)]

                def emit_blocks(b0, b1):
                    for bi in range(b0, b1):
                        bo, bs = nblk[bi]
                        sl = np.s_[:, bo:bo + bs]
                        zs = zspool.tile([128, 512], bf, tag="zs")
                        if not cfg.SELF_EDGES:
                            # self-loop term: z += x_tilde^T pre dst scale
                            nc.vector.tensor_tensor(zb[bi][:, :bs],
                                                    zb[bi][:, :bs],
                                                    xt_prev[sl],
                                                    mybir.AluOpType.add)
                        nc.vector.tensor_tensor(zs[:, :bs], zb[bi][:, :bs],
                                                dinvbc_sb[sl],
                                                mybir.AluOpType.mult)
                        psw = psw_pool.tile([cout, 512], f32, tag="psW")
                        nc.tensor.matmul(psw[:, :bs], w_sb[layer][:],
                                         zs[:, :bs], start=True, stop=True)
                        if layer < 2:
                            tmp = zspool.tile([128, 512], bf, tag="acttmp")
                            nc.scalar.activation(
                                tmp[:, :bs], psw[:, :bs],
                                mybir.ActivationFunctionType.Relu,
                                bias=b_sb[layer][:])
                            nc.vector.tensor_tensor(xt[sl], tmp[:, :bs],
                                                    dinvbc_sb[sl],
                                                    mybir.AluOpType.mult)
                        else:
                            nc.scalar.activation(
                                fin[sl], psw[:cout, :bs],
                                mybir.ActivationFunctionType.Identity,
                                bias=b_sb[layer][:])

                def emit_epilogue(t0, t1):
                    # transpose 64xN^T -> node-major fp32, DMA out
                    nfull = cfg.NPC // 128
                    rem = cfg.NPC - nfull * 128
                    for t in range(t0, min(t1, nfull + (1 if rem else 0))):
                        tp = pst.tile([128, OUT_C], f32, tag="psT")
                        nc.tensor.transpose(
                            tp[:], fin[:, t * 128:(t + 1) * 128], ident[:])
                        nc.scalar.copy(out_nm[:, t, :], tp[:])
                    d0, d1 = t0, min(t1, nfull)
                    if d1 > d0:
                        nc.sync.dma_start(
                            out_d[d0 * 128:d1 * 128]
                            .rearrange("(t p) c -> p t c", p=128),
                            out_nm[:, d0:d1, :])
                    if t1 >= tpc and rem:
                        nc.sync.dma_start(out_d[nfull * 128:cfg.NPC],
                                          out_nm[:rem, nfull, :])

                # ---- one-hot S blocks generated on-chip: S[e, p] =
                # (dpos[e, ch] == iota[p]), SBATCH channels per DVE op.
                for b in range(nblk_s):
                    s = spool.tile([128, cfg.SBATCH, 128], bf, tag="s")
                    d_ap = dpos_sb[:, b * cfg.SBATCH:(b + 1) * cfg.SBATCH] \
                        .unsqueeze(2).broadcast_to([128, cfg.SBATCH, 128])
                    i_ap = iota_sb[:].unsqueeze(1) \
                        .broadcast_to([128, cfg.SBATCH, 128])
                    nc.vector.tensor_tensor(s[:], d_ap, i_ap,
                                            mybir.AluOpType.is_equal)
                    for k in range(cfg.SBATCH):
                        s_slots.append((s, k))
                s_slots = s_slots[:nch]

                # ---- interleaved emission: lo gathers+matmuls; hi
                # gathers+matmuls through TSPLIT; A-half tail + AllGather;
                # then the remaining hi stream and the B-half tail.
                emit_gathers(idxlo_sb, xt_fullA[layer][:], 0, lo_total)
                emit_mm(0, clo, 0, tpc)
                emit_gathers(idxhi_sb, xt_fullB[layer][:], 0, hi_split)
                emit_mm(1, chi, 0, TSPLIT)
                emit_blocks(0, BSPLIT)
                if layer < 2:
                    nmA = nmpool.tile([128, ta, C], bf, tag="nm")
                    nc.sync.dma_start_transpose(nmA[:], xt[:, :spA])
                    emit_ag_half(layer + 1, 0, nmA[:])
                else:
                    emit_epilogue(0, TSPLIT)
                emit_gathers(idxhi_sb, xt_fullB[layer][:], hi_split,
                             hi_total)
                emit_mm(1, chi, TSPLIT, tpc)
                emit_blocks(BSPLIT, nzb)
                assert gi[0] == nch

                if layer < 2:
                    nmB = nmpool.tile([128, tpc - ta, C], bf, tag="nm")
                    nc.sync.dma_start_transpose(nmB[:], xt[:, spA:])
                    emit_ag_half(layer + 1, 1, nmB[:])
                    xt_prev = xt
                else:
                    emit_epilogue(TSPLIT, tpc)

    nc.compile()
    return nc


# ------------------------------------------------------------------ driver
_CACHE = {}


def _get_program(cfg, plan):
    key = (cfg.N, cfg.NCORES, tuple(plan["clo"]), tuple(plan["chi"]))
    if key not in _CACHE:
        _CACHE[key] = _build(cfg, plan)
    return _CACHE[key]


def _make_in_maps(cfg, x, weights, biases, plan, per_core, dinv):
    x = np.asarray(x, dtype=np.float32)
    npc, npcp, spA = cfg.NPC, cfg.NPC_PAD, cfg.SPLITA

    iota_bc = np.tile(np.arange(128, dtype=np.float32), (128, 1)).astype(BF16)
    idx_dummy = np.tile(np.arange(128, dtype=np.int16).reshape(-1, 16).T,
                        (8, 1)).copy()
    # host-side prologue: x_tilde = dinv * x, padded per core, split into
    # the replicated A/B gather tables + per-core channel-major own slice
    xt_pad = np.zeros((cfg.NCORES, npcp, cfg.C), dtype=BF16)
    xtf = (x * dinv[:cfg.N, None]).astype(BF16)
    for c in range(cfg.NCORES):
        xt_pad[c, :npc] = xtf[c * npc:(c + 1) * npc]
    ta, tb = cfg.TA, cfg.TPC - cfg.TA
    # p-major row order within each core block (row = p*T + t)
    xt0A = np.ascontiguousarray(
        xt_pad[:, :spA].reshape(cfg.NCORES, ta, 128, cfg.C)
        .transpose(0, 2, 1, 3).reshape(cfg.ROWSA, cfg.C))
    xt0B = np.ascontiguousarray(
        xt_pad[:, spA:].reshape(cfg.NCORES, tb, 128, cfg.C)
        .transpose(0, 2, 1, 3).reshape(cfg.ROWSB, cfg.C))
    in_maps = []
    for c in range(cfg.NCORES):
        m = {
            "xt0A": xt0A,
            "xt0B": xt0B,
            "xtprev0": np.ascontiguousarray(xt_pad[c].T),
            "idx_lo": per_core[c]["idx_lo"],
            "idx_hi": per_core[c]["idx_hi"],
            "dpos_t": per_core[c]["dpos_t"],
            "iota_bc": iota_bc,
            "idx_dummy": idx_dummy,
            "dinv_bc": per_core[c]["dinv_bc"],
        }
        for i in range(3):
            m[f"w{i}"] = np.asarray(weights[i], dtype=np.float32) \
                .astype(BF16)
            m[f"b{i}"] = np.asarray(biases[i], dtype=np.float32) \
                .reshape(-1, 1)
        in_maps.append(m)
    return in_maps


def run(cfg, x, edge_index, weights, biases, sim=False, trace=False):
    plan, per_core, dinv = _preprocess(cfg, edge_index)
    nc = _get_program(cfg, plan)
    in_maps = _make_in_maps(cfg, x, weights, biases, plan, per_core, dinv)

    if sim:
        from concourse.bass_interp import MultiCoreSim

        s = MultiCoreSim(nc, num_cores=cfg.NCORES, num_workers=1)
        for c in range(cfg.NCORES):
            for k, v in in_maps[c].items():
                s.cores[c].tensor(k)[:] = v
        s.simulate()
        results = [{"out": s.cores[c].tensor("out").copy()}
                   for c in range(cfg.NCORES)]
        res = None
    else:
        _install_ntff_hook()
        res = bass_utils.run_bass_kernel_spmd(
            nc, in_maps, core_ids=list(range(cfg.NCORES)), trace=trace)
        results = res.results

    out = np.concatenate([results[c]["out"] for c in range(cfg.NCORES)], 0)
    return out, res


def kernel(x, edge_index, W1, b1, W2, b2, W3, b3):
    out, _ = run(FULL, x, edge_index, (W1, W2, W3), (b1, b2, b3))
    return out

